# revision 15
# baseline (speedup 1.0000x reference)
"""ConfusionPenaltyLoss Trainium2 kernel.

Reference computation (B=4096, T=128, C=37, L=8):
  positions = floor(linspace(0, T-1, L)) = [0,18,36,54,72,90,108,127]
  lp  = log_probs[:, positions, :]           # [B, L, C]
  tgt = targets.reshape(B, L)
  W[b,l,c] = mask[tgt[b,l], c]  (one-hot of partner(gt) for the 8 symmetric
             confusion pairs, else all-zero row)
  total = sum(W * exp(lp)) * 3.0 ; n = sum(W) ; out = total/n (0 if n==0)

Strategy: data-parallel over batch across 8 NeuronCores (512 batches/core).
Per core, 4096 (b,l) rows live at [partition = row//32, slot = row%32];
slot s = bl*8 + l with b = ph*4 + bl.

W depends only on targets, so the host uploads it directly as a dense
one-hot f32 mask M [128, 32*37] (a single 4.7KB/partition sequential DMA
— cheap next to the scattered gather). n = sum(W) is likewise computed
on host. The device computes only sum(M * exp(LP)):
  per bl-quarter: ACT exp (as soon as that quarter's gather lands), then
  one fused DVE tensor_tensor_reduce (mult + add-accumulate) chaining a
  per-partition accumulator. Scalar ships the final 128 partials.

DMA: the gather needs 4096 scattered 148-byte rows/core — descriptor-rate
bound on the 16 shared DMA engines (~3.5us floor). Both HWDGE queues
(sync + scalar) each carry two bl quarters, interleaved uniform/t=127 so
each quarter completes as early as possible. (No gpsimd SWDGE: it makes
the NEFF teardown's sem sweep stall for multiple us on random sems.)
"""

import numpy as np

NUM_CLASSES = 37
PENALTY_SCALE = 3.0
CONFUSION_PAIRS = [(1, 25), (2, 35), (5, 28), (8, 11), (13, 22), (6, 16), (9, 17), (3, 12)]
PARTNER = {}
for _a, _b in CONFUSION_PAIRS:
    PARTNER[_a] = _b
    PARTNER[_b] = _a

B, T, C, L = 4096, 128, 37, 8
N_CORES = 8
BS = B // N_CORES            # 512 batches per core
ROWS = BS * L                # 4096 (b,l) rows per core
SLOTS = ROWS // 128          # 32 row-slots per partition
LU = 7                       # uniform positions 0,18,...,108 (stride 18)

_CACHE = {}


def _build_nc():
    from contextlib import ExitStack

    from concourse import bacc, mybir

    f32 = mybir.dt.float32
    Alu = mybir.AluOpType

    nc = bacc.Bacc("TRN2", target_bir_lowering=False, debug=False, num_devices=N_CORES)

    lp = nc.dram_tensor("lp", [BS, T, C], f32, kind="ExternalInput").ap()
    m_in = nc.dram_tensor("m", [128, SLOTS * C], f32, kind="ExternalInput").ap()
    out = nc.dram_tensor("out", [128, 1], f32, kind="ExternalOutput").ap()

    with ExitStack() as ctx:
        sb = lambda name, shape, dt: ctx.enter_context(
            nc.sbuf_tensor(name, shape, dt)
        ).ap()
        LP = sb("LP", [128, SLOTS * C], f32)
        M = sb("M", [128, SLOTS * C], f32)
        ELP = sb("ELP", [128, SLOTS * C], f32)
        TTO = sb("TTO", [128, SLOTS * C], f32)
        TOT = sb("TOT", [128, 4], f32)

        s_q1 = ctx.enter_context(nc.semaphore("s_q1"))
        s_q10 = ctx.enter_context(nc.semaphore("s_q10"))
        s_exp = ctx.enter_context(nc.semaphore("s_exp"))
        s_ttr = ctx.enter_context(nc.semaphore("s_ttr"))
        s_outdma = ctx.enter_context(nc.semaphore("s_outdma"))

        # DRAM views. b = ph*4 + bl.
        lp4 = lp.rearrange("(ph bl) t c -> ph bl t c", bl=4)
        # uniform positions as one affine view: t = lu*18, lu = 0..6
        lpu = lp[:, 0 : LU * 18, :].rearrange(
            "(ph bl) (lu x) c -> ph bl lu x c", bl=4, x=18
        )[:, :, :, 0, :]
        LPv = LP.rearrange("p (bl l c) -> p bl l c", bl=4, l=L)
        LPq = LP.rearrange("p (bl lc) -> p bl lc", bl=4)
        Mq = M.rearrange("p (bl lc) -> p bl lc", bl=4)
        ELPq = ELP.rearrange("p (bl lc) -> p bl lc", bl=4)
        TTOq = TTO.rearrange("p (bl lc) -> p bl lc", bl=4)

        # exp/accumulate order: quarters sorted by expected gather
        # completion (each queue drains its posts in order)
        QORDER = [0, 2, 1, 3]
        QWAIT = {0: (s_q1, 48), 1: (s_q1, 80), 2: (s_q10, 32), 3: (s_q10, 64)}

        with nc.Block() as block:

            @block.sync
            def _(sync):
                # queue 1: mask first (needed by the first TTR), then
                # bl 0:2 interleaved uniform / t=127 so bl0 lands early.
                # DMA APs max 3 dims incl partition -> per-bl posts.
                sync.dma_start(out=M[:], in_=m_in).then_inc(s_q1, 16)
                for bl in range(2):
                    sync.dma_start(
                        out=LPv[:, bl, 0:LU, :], in_=lpu[:, bl]
                    ).then_inc(s_q1, 16)
                    sync.dma_start(
                        out=LPv[:, bl, LU:L, :], in_=lp4[:, bl, T - 1 : T, :]
                    ).then_inc(s_q1, 16)

            @block.scalar
            def _(scalar):
                # queue 10: bl 2:4
                for bl in range(2, 4):
                    scalar.dma_start(
                        out=LPv[:, bl, 0:LU, :], in_=lpu[:, bl]
                    ).then_inc(s_q10, 16)
                    scalar.dma_start(
                        out=LPv[:, bl, LU:L, :], in_=lp4[:, bl, T - 1 : T, :]
                    ).then_inc(s_q10, 16)
                for i, q in enumerate(QORDER):
                    sem, val = QWAIT[q]
                    scalar.wait_ge(sem, val)
                    scalar.activation(
                        out=ELPq[:, q],
                        in_=LPq[:, q],
                        func=mybir.ActivationFunctionType.Exp,
                    ).then_inc(s_exp, 1)
                # ship the 128 partials once the accumulator chain ends.
                # No receipt wait: NEFF teardown far outlasts the write.
                scalar.wait_ge(s_ttr, 1)
                scalar.dma_start(out=out, in_=TOT[:, 0:1]).then_inc(s_outdma, 16)

            @block.vector
            def _(vector):
                # TOT[:,i] = sum over the quarter of M_q * exp(LP_q)
                vector.wait_ge(s_q1, 16)  # mask landed
                for i, q in enumerate(QORDER):
                    vector.wait_ge(s_exp, i + 1)
                    vector.tensor_tensor(
                        out=TTOq[:, q], in0=ELPq[:, q], in1=Mq[:, q], op=Alu.mult
                    )
                last = vector.tensor_reduce(
                    out=TOT[:, 0:1],
                    in_=TTO[:],
                    axis=mybir.AxisListType.X,
                    op=Alu.add,
                )
                last.then_inc(s_ttr, 1)

    nc.compile()
    return nc


def _get_nc():
    if "nc" not in _CACHE:
        _CACHE["nc"] = _build_nc()
    return _CACHE["nc"]


def _prep(log_probs, targets):
    lp = np.ascontiguousarray(np.asarray(log_probs, dtype=np.float32))
    tg = np.ascontiguousarray(np.asarray(targets).astype(np.int64))
    # dense one-hot partner mask, [ROWS, C] per core; count = paired rows
    m_full = np.zeros((N_CORES * ROWS, C), dtype=np.float32)
    for g, p in PARTNER.items():
        m_full[tg == g, p] = 1.0
    count = int(np.isin(tg, list(PARTNER)).sum())
    in_maps = []
    for i in range(N_CORES):
        rows = slice(i * ROWS, (i + 1) * ROWS)
        in_maps.append(
            {
                "lp": lp[i * BS : (i + 1) * BS],
                "m": m_full[rows].reshape(128, SLOTS * C),
            }
        )
    return in_maps, count


def kernel(log_probs, targets, target_lengths, **_kwargs):
    from concourse.bass_utils import run_bass_kernel_spmd

    nc = _get_nc()
    in_maps, count = _prep(log_probs, targets)
    res = run_bass_kernel_spmd(
        nc, in_maps, list(range(N_CORES)), **_CACHE.get("run_kwargs", {})
    )
    _CACHE["last_result"] = res
    total = sum(
        float(np.asarray(r["out"], dtype=np.float64).sum()) for r in res.results
    )
    if count > 0:
        return np.array(PENALTY_SCALE * total / count, dtype=np.float32)
    return np.array(0.0, dtype=np.float32)


# revision 16
# speedup vs baseline: 1.0630x; 1.0630x over previous
"""ConfusionPenaltyLoss Trainium2 kernel.

Reference computation (B=4096, T=128, C=37, L=8):
  positions = floor(linspace(0, T-1, L)) = [0,18,36,54,72,90,108,127]
  lp  = log_probs[:, positions, :]           # [B, L, C]
  tgt = targets.reshape(B, L)
  W[b,l,c] = mask[tgt[b,l], c]  (one-hot of partner(gt) for the 8 symmetric
             confusion pairs, else all-zero row)
  total = sum(W * exp(lp)) * 3.0 ; n = sum(W) ; out = total/n (0 if n==0)

Strategy: data-parallel over batch across 8 NeuronCores (512 batches/core).
Per core, 4096 (b,l) rows live at [partition = row//32, slot = row%32];
slot s = bl*8 + l with b = ph*4 + bl.

Each class belongs to at most one pair, so W is one-hot per row:
  device   s[row] = sum_k (tgt[row]==a_k) * lp[row, b_k]   (16 small
           scalar_tensor_tensor ops over the [128,32] row tile, one per
           ordered pair; k-sum via one tensor_reduce)
  device   out[p] = sum_s exp(s[p,s])  (one ACT exp with accum_out, and
           the Scalar engine ships the 128 partials itself)
Unpaired rows contribute exp(0)=1 each; the host subtracts their exact
count. n = number of paired rows — also computed on host from targets.

DMA: the gather needs 4096 scattered 148-byte rows/core. The 16 shared
DMA engines move ~10GB/s each regardless of chunk size, so the floor is
~3.5us and nothing else may ride the queues (a dense f32 mask upload,
tried and reverted, doubled the traffic and cost 2us). Both HWDGE
queues (sync + scalar) carry one batch-half each: the 7 uniform
positions (t = 0..108 step 18) post per-bl as one 3D-AP each, plus one
post for t=127. Targets (16KB) ride first on the sync queue. No gpsimd
SWDGE — leaving it out keeps the NEFF teardown sweep shorter on average
(it stalls multi-us on random semaphore resets in some configurations).
"""

import numpy as np

NUM_CLASSES = 37
PENALTY_SCALE = 3.0
CONFUSION_PAIRS = [(1, 25), (2, 35), (5, 28), (8, 11), (13, 22), (6, 16), (9, 17), (3, 12)]
ORDERED_PAIRS = [(a, b) for a, b in CONFUSION_PAIRS] + [(b, a) for a, b in CONFUSION_PAIRS]
PAIRED_SET = sorted({a for a, _ in ORDERED_PAIRS})

B, T, C, L = 4096, 128, 37, 8
N_CORES = 8
BS = B // N_CORES            # 512 batches per core
ROWS = BS * L                # 4096 (b,l) rows per core
SLOTS = ROWS // 128          # 32 row-slots per partition
LU = 7                       # uniform positions 0,18,...,108 (stride 18)

_CACHE = {}


def _build_nc():
    from contextlib import ExitStack

    from concourse import bacc, mybir

    f32 = mybir.dt.float32
    Alu = mybir.AluOpType

    nc = bacc.Bacc("TRN2", target_bir_lowering=False, debug=False, num_devices=N_CORES)

    lp = nc.dram_tensor("lp", [BS, T, C], f32, kind="ExternalInput").ap()
    tgc = nc.dram_tensor("tgc", [128, SLOTS], f32, kind="ExternalInput").ap()
    out = nc.dram_tensor("out", [128, 1], f32, kind="ExternalOutput").ap()

    with ExitStack() as ctx:
        sb = lambda name, shape, dt: ctx.enter_context(
            nc.sbuf_tensor(name, shape, dt)
        ).ap()
        LP = sb("LP", [128, SLOTS * C], f32)
        TT = sb("TT", [128, SLOTS], f32)
        SEL = sb("SEL", [128, SLOTS * 16], f32)
        S1 = sb("S1", [128, SLOTS], f32)
        E = sb("E", [128, SLOTS], f32)
        OUTT = sb("OUTT", [128, 1], f32)

        s_tgc = ctx.enter_context(nc.semaphore("s_tgc"))
        s_lp = ctx.enter_context(nc.semaphore("s_lp"))
        s_s1 = ctx.enter_context(nc.semaphore("s_s1"))
        s_act = ctx.enter_context(nc.semaphore("s_act"))
        s_outdma = ctx.enter_context(nc.semaphore("s_outdma"))

        # DRAM views. b = ph*4 + bl.
        lp4 = lp.rearrange("(ph bl) t c -> ph bl t c", bl=4)
        # uniform positions as one affine view: t = lu*18, lu = 0..6
        lpu = lp[:, 0 : LU * 18, :].rearrange(
            "(ph bl) (lu x) c -> ph bl lu x c", bl=4, x=18
        )[:, :, :, 0, :]
        LPv = LP.rearrange("p (bl l c) -> p bl l c", bl=4, l=L)
        LPS = LP.rearrange("p (s c) -> p s c", c=C)
        SEL3 = SEL.rearrange("p (s k) -> p s k", k=16)

        with nc.Block() as block:

            @block.sync
            def _(sync):
                # queue 1: targets (tiny) then batch-half bl 0:2. DMA APs
                # max 3 dims incl partition -> uniform gather posts per-bl.
                sync.dma_start(out=TT[:], in_=tgc).then_inc(s_tgc, 16)
                for bl in range(2):
                    sync.dma_start(
                        out=LPv[:, bl, 0:LU, :], in_=lpu[:, bl]
                    ).then_inc(s_lp, 16)
                sync.dma_start(
                    out=LPv[:, 0:2, LU:L, :], in_=lp4[:, 0:2, T - 1 : T, :]
                ).then_inc(s_lp, 16)

            @block.scalar
            def _(scalar):
                # queue 10: batch-half bl 2:4
                for bl in range(2, 4):
                    scalar.dma_start(
                        out=LPv[:, bl, 0:LU, :], in_=lpu[:, bl]
                    ).then_inc(s_lp, 16)
                scalar.dma_start(
                    out=LPv[:, 2:4, LU:L, :], in_=lp4[:, 2:4, T - 1 : T, :]
                ).then_inc(s_lp, 16)
                # exp + per-partition row-sum in one op, then ship the
                # 128 partials. s_act orders the ring write after the exp
                # completes (the engine otherwise issues it while ACT is
                # still draining). No receipt wait: NEFF teardown far
                # outlasts the 512B write.
                scalar.wait_ge(s_s1, 1)
                scalar.activation(
                    out=E[:],
                    in_=S1[:],
                    func=mybir.ActivationFunctionType.Exp,
                    accum_out=OUTT[:, 0:1],
                ).then_inc(s_act, 1)
                scalar.wait_ge(s_act, 1)
                scalar.dma_start(out=out, in_=OUTT[:]).then_inc(s_outdma, 16)

            @block.vector
            def _(vector):
                # s[row] = sum_k (tgt==a_k) * lp[row, b_k]. The
                # same-engine SEL->reduce RAW is safe without a sem: the
                # reduce reads element (s,k) later than the k-th SEL
                # wrote it, with a >=200ns head start.
                vector.wait_ge(s_tgc, 16)
                vector.wait_ge(s_lp, 96)
                for k, (a, b) in enumerate(ORDERED_PAIRS):
                    vector.scalar_tensor_tensor(
                        out=SEL3[:, :, k],
                        in0=TT[:],
                        scalar=float(a),
                        in1=LPS[:, :, b],
                        op0=Alu.is_equal,
                        op1=Alu.mult,
                    )
                vector.tensor_reduce(
                    out=S1[:], in_=SEL3, axis=mybir.AxisListType.X, op=Alu.add
                ).then_inc(s_s1, 1)

    nc.compile()
    return nc


def _get_nc():
    if "nc" not in _CACHE:
        _CACHE["nc"] = _build_nc()
    return _CACHE["nc"]


def _prep(log_probs, targets):
    lp = np.ascontiguousarray(np.asarray(log_probs, dtype=np.float32))
    tg = np.ascontiguousarray(np.asarray(targets).astype(np.int64))
    paired = np.isin(tg, PAIRED_SET)
    in_maps = []
    unpaired_counts = []
    for i in range(N_CORES):
        rows = slice(i * ROWS, (i + 1) * ROWS)
        in_maps.append(
            {
                "lp": lp[i * BS : (i + 1) * BS],
                "tgc": tg[rows].reshape(128, SLOTS).astype(np.float32),
            }
        )
        unpaired_counts.append(ROWS - int(paired[rows].sum()))
    return in_maps, unpaired_counts, int(paired.sum())


def kernel(log_probs, targets, target_lengths, **_kwargs):
    from concourse.bass_utils import run_bass_kernel_spmd

    nc = _get_nc()
    in_maps, unpaired_counts, count = _prep(log_probs, targets)
    res = run_bass_kernel_spmd(
        nc, in_maps, list(range(N_CORES)), **_CACHE.get("run_kwargs", {})
    )
    _CACHE["last_result"] = res
    total = 0.0
    for r, unp in zip(res.results, unpaired_counts):
        total += float(np.asarray(r["out"], dtype=np.float64).sum()) - unp
    if count > 0:
        return np.array(PENALTY_SCALE * total / count, dtype=np.float32)
    return np.array(0.0, dtype=np.float32)
